# revision 5
# baseline (speedup 1.0000x reference)
"""Atomwise (segment_reduce) Trainium2 kernel.

y[m] = sum_{atoms i in molecule m} (x[i] . W[0] + b[0]),  m in [0, 100000)

Strategy (8 NeuronCores, SPMD, no collectives needed):
  - Host cuts the (sorted) atom axis at molecule boundaries into 8 nearly
    equal shards -> each core owns a disjoint, contiguous molecule range.
  - Per core, molecules are processed in chunks of 128 (the PSUM partition
    dim).  The host materializes, per chunk, a fixed-size window of A_max
    atom rows (the chunk's atoms, zero-padded) so every core runs the SAME
    static graph.
  - Device per chunk:
      * DMA the [A_max, 128] f32 window as NB tiles of [128 atoms, 128 feat]
      * ScalarE casts f32 -> bf16 into a [128, NB, 129] layout whose last
        column per block is memset to 1.0 (counts column for the bias term)
      * VectorE builds the one-hot matrix H[a, j] = (lidx[a] == j) in bf16
        with a single is_equal over the whole chunk (lidx broadcast vs iota)
      * TensorE accumulates S_aug[128 mols, 129] = sum_b H_b^T @ Xaug_b in
        PSUM (the segment-sum of atom feature rows + per-molecule counts)
      * VectorE: y_chunk[m] = sum_f S_aug[m, f] * w0aug[f]  where
        w0aug = [W[0,:], b[0]]  (tensor_tensor_reduce, one op)
      * DMA y_chunk out.
  - Host stitches the 8 disjoint per-core molecule ranges into y[100000].
"""

import numpy as np
import ml_dtypes

N_ATOMS = 2_000_000
N_IN = 128
N_MOL = 100_000
NCORES = 8
P = 128
NFA = N_IN + 1  # 128 features + 1 counts column

_graph_cache: dict = {}


def _build_graph(NCH: int, NB: int):
    import concourse.mybir as mybir
    from concourse import bacc
    from concourse.tile import TileContext

    f32 = mybir.dt.float32
    bf16 = mybir.dt.bfloat16
    A_max = NB * P
    IOTA_OFF = 0
    LIDX_OFF = NB * P
    W0_OFF = ((LIDX_OFF + NCH * NB + 1) // 2) * 2
    CW = W0_OFF + 2 * NFA

    nc = bacc.Bacc()
    xw = nc.dram_tensor("xw", [NCH * A_max, N_IN], f32, kind="ExternalInput")
    cst = nc.dram_tensor("cst", [P, CW], bf16, kind="ExternalInput")
    out = nc.dram_tensor("out", [NCH * P], f32, kind="ExternalOutput")

    xw_r = xw.rearrange("(c b p) f -> c p b f", b=NB, p=P)
    out_r = out.rearrange("(c p f) -> c p f", p=P, f=1)

    with TileContext(nc) as tc:
        with tc.tile_pool(name="const", bufs=1) as cpool, \
             tc.tile_pool(name="xf", bufs=3) as xpool, \
             tc.tile_pool(name="xbp", bufs=3) as xbpool, \
             tc.tile_pool(name="hp", bufs=3) as hpool, \
             tc.tile_pool(name="ep", bufs=2) as epool, \
             tc.tile_pool(name="pp", bufs=4, space="PSUM") as pspool:
            cst_t = cpool.tile([P, CW], bf16)
            nc.sync.dma_start(cst_t[:], cst[:, :])
            iota3 = cst_t[:, IOTA_OFF:IOTA_OFF + NB * P].rearrange(
                "p (b f) -> p b f", b=NB)
            w0_t = cst_t[:, W0_OFF:W0_OFF + 2 * NFA].bitcast(f32)

            for c in range(NCH):
                xt = xpool.tile([P, NB * N_IN], f32, tag="x")
                nc.sync.dma_start(
                    xt[:].rearrange("p (b f) -> p b f", b=NB), xw_r[c]
                )
                xb = xbpool.tile([P, NB * NFA], bf16, tag="xb")
                xb3 = xb[:].rearrange("p (b f) -> p b f", b=NB)
                nc.vector.memset(xb3[:, :, N_IN:NFA], 1.0)
                nc.scalar.activation(
                    xb3[:, :, 0:N_IN],
                    xt[:].rearrange("p (b f) -> p b f", b=NB),
                    mybir.ActivationFunctionType.Copy,
                )
                ht = hpool.tile([P, NB * P], bf16, tag="h")
                nc.vector.tensor_tensor(
                    out=ht[:].rearrange("p (b f) -> p b f", b=NB),
                    in0=cst_t[:, LIDX_OFF + c * NB:LIDX_OFF + (c + 1) * NB
                              ].to_broadcast([P, NB, P]),
                    in1=iota3,
                    op=mybir.AluOpType.is_equal,
                )
                ps = pspool.tile([P, NFA], f32, tag="ps")
                for bb in range(NB):
                    nc.tensor.matmul(
                        ps[:],
                        lhsT=ht[:, bb * P:(bb + 1) * P],
                        rhs=xb[:, bb * NFA:(bb + 1) * NFA],
                        start=(bb == 0),
                        stop=(bb == NB - 1),
                    )
                prod = epool.tile([P, NFA], f32, tag="prod")
                yc = epool.tile([P, 1], f32, tag="yc")
                nc.vector.scalar_tensor_tensor(
                    out=prod[:],
                    in0=ps[:],
                    scalar=1.0,
                    in1=w0_t[:],
                    op0=mybir.AluOpType.mult,
                    op1=mybir.AluOpType.mult,
                    accum_out=yc[:],
                )
                nc.sync.dma_start(out_r[c], yc[:])
    nc.finalize()
    return nc


def _prep(inputs):
    x = np.ascontiguousarray(np.asarray(inputs["scalar_representation"], dtype=np.float32))
    idx = np.asarray(inputs["idx_m"]).astype(np.int64)
    W = np.asarray(inputs["W"], dtype=np.float32)
    b = np.asarray(inputs["b"], dtype=np.float32)
    n = x.shape[0]

    # mol_start[m] = first atom index belonging to molecule m (m in 0..N_MOL)
    mol_start = np.searchsorted(idx, np.arange(N_MOL + 1), side="left")

    # Cut cores at molecule boundaries near equal-atom splits
    targets = (np.arange(NCORES + 1) * n) // NCORES
    mcut = np.searchsorted(mol_start, targets, side="left").astype(np.int64)
    mcut[0], mcut[-1] = 0, N_MOL

    m_counts = np.diff(mcut)
    NCH = int(np.ceil(m_counts.max() / P))

    # Max atoms spanned by any 128-molecule chunk on any core
    span_max = 0
    core_chunks = []  # per core: list of (astart, aend, gm)
    for i in range(NCORES):
        chunks = []
        for c in range(NCH):
            gm = mcut[i] + c * P
            gm_end = min(gm + P, mcut[i + 1])
            if gm >= mcut[i + 1]:
                chunks.append((0, 0, 0))
                continue
            astart = int(mol_start[gm])
            aend = int(mol_start[gm_end])
            chunks.append((astart, aend, int(gm)))
            span_max = max(span_max, aend - astart)
        core_chunks.append(chunks)
    NB = max(1, int(np.ceil(span_max / P)))
    A_max = NB * P

    in_maps = []
    IOTA_OFF = 0
    LIDX_OFF = NB * P
    W0_OFF = ((LIDX_OFF + NCH * NB + 1) // 2) * 2
    CW = W0_OFF + 2 * NFA
    iota_row = np.tile(np.arange(P, dtype=np.float32), NB).astype(ml_dtypes.bfloat16)
    w0aug_row = np.concatenate([W[0], b[0:1]]).astype(np.float32).view(ml_dtypes.bfloat16)

    for i in range(NCORES):
        xw_i = np.zeros((NCH * A_max, N_IN), dtype=np.float32)
        lidx_flat = np.full(NCH * A_max, -1.0, dtype=np.float32)
        for c, (astart, aend, gm) in enumerate(core_chunks[i]):
            spn = aend - astart
            if spn <= 0:
                continue
            xw_i[c * A_max:c * A_max + spn] = x[astart:aend]
            lidx_flat[c * A_max:c * A_max + spn] = idx[astart:aend] - gm
        # layout [P, NCH*NB]: col (c*NB + b), row p  -> atom (c, b*128 + p)
        lidx_t = lidx_flat.reshape(NCH * NB, P).T.astype(ml_dtypes.bfloat16)
        cst = np.zeros((P, CW), dtype=ml_dtypes.bfloat16)
        cst[:, IOTA_OFF:IOTA_OFF + NB * P] = iota_row[None, :]
        cst[:, LIDX_OFF:LIDX_OFF + NCH * NB] = lidx_t
        cst[:, W0_OFF:W0_OFF + 2 * NFA] = w0aug_row[None, :]
        in_maps.append({"xw": xw_i, "cst": np.ascontiguousarray(cst)})
    return in_maps, mcut, m_counts, NCH, NB


def _run(inputs, trace=False):
    from concourse import bass_utils

    in_maps, mcut, m_counts, NCH, NB = _prep(inputs)
    key = (NCH, NB)
    if key not in _graph_cache:
        _graph_cache[key] = _build_graph(NCH, NB)
    nc = _graph_cache[key]

    res = bass_utils.run_bass_kernel_spmd(
        nc, in_maps, core_ids=list(range(NCORES)), trace=trace
    )
    y = np.zeros(N_MOL, dtype=np.float32)
    for i in range(NCORES):
        mc = int(m_counts[i])
        y[mcut[i]:mcut[i] + mc] = res.results[i]["out"][:mc]
    return y, res


def kernel(**inputs) -> np.ndarray:
    y, _ = _run(inputs, trace=False)
    return y


# revision 6
# speedup vs baseline: 3.3560x; 3.3560x over previous
"""Atomwise (segment_reduce) Trainium2 kernel.

y[m] = sum_{atoms i in molecule m} (x[i] . W[0] + b[0]),  m in [0, 100000)

Strategy (8 NeuronCores, SPMD, no collectives needed):
  - Host cuts the (sorted) atom axis at molecule boundaries into 8 nearly
    equal shards -> each core owns a disjoint, contiguous molecule range.
  - Per core, molecules are processed in chunks of 128 (the PSUM partition
    dim).  The host materializes, per chunk, a fixed-size window of A_max
    atom rows in bf16 with a ones-column appended (counts column for the
    bias term), so every core runs the SAME static graph and each DMA
    descriptor moves one contiguous ~5.7KB run per partition.
  - Atom->partition mapping is p-major: partition p holds the chunk's
    atoms [p*NB, (p+1)*NB) as NB row-groups of 129 values (128 feat + 1).
  - Device per chunk:
      * one DMA of the [128, NB*129] bf16 tile
      * VectorE builds one-hot H[p, j, m] = (lidx[p, j] == m) in bf16 with
        a single is_equal over the whole chunk (lidx broadcast vs iota)
      * TensorE accumulates S_aug[128 mols, 129] = sum_j H_j^T @ Xaug_j in
        PSUM (segment-sum of atom feature rows + per-molecule counts)
      * VectorE: y_all[m, c] = sum_f S_aug[m, f] * w0aug[f]  where
        w0aug = [W[0,:], b[0]]  (scalar_tensor_tensor with accum_out)
  - One output DMA of y_all [128, NCH] at the end; host un-permutes and
    stitches the 8 disjoint per-core molecule ranges into y[100000].
"""

import numpy as np
import ml_dtypes

N_ATOMS = 2_000_000
N_IN = 128
N_MOL = 100_000
NCORES = 8
P = 128
NFA = N_IN + 1  # 128 features + 1 counts column

_graph_cache: dict = {}


def _build_graph(NCH: int, NB: int):
    import concourse.mybir as mybir
    from concourse import bacc
    from concourse.tile import TileContext

    f32 = mybir.dt.float32
    bf16 = mybir.dt.bfloat16
    A_max = NB * P
    IOTA_OFF = 0
    LIDX_OFF = NB * P
    W0_OFF = ((LIDX_OFF + NCH * NB + 1) // 2) * 2
    CW = W0_OFF + 2 * NFA

    nc = bacc.Bacc()
    xw = nc.dram_tensor("xw", [NCH * A_max, NFA], bf16, kind="ExternalInput")
    cst = nc.dram_tensor("cst", [P, CW], bf16, kind="ExternalInput")
    out = nc.dram_tensor("out", [P * NCH], f32, kind="ExternalOutput")

    # row (c, p, j) -> partition p, free (j*NFA + f): per-partition
    # contiguous NB*NFA*2 bytes in DRAM per chunk
    xw_r = xw.rearrange("(c p j) f -> c p (j f)", p=P, j=NB)
    out_r = out.rearrange("(p c) -> p c", c=NCH)

    with TileContext(nc) as tc:
        with tc.tile_pool(name="const", bufs=1) as cpool, \
             tc.tile_pool(name="xbp", bufs=4) as xbpool, \
             tc.tile_pool(name="hp", bufs=3) as hpool, \
             tc.tile_pool(name="ep", bufs=2) as epool, \
             tc.tile_pool(name="pp", bufs=4, space="PSUM") as pspool:
            cst_t = cpool.tile([P, CW], bf16)
            nc.sync.dma_start(cst_t[:], cst[:, :])
            iota3 = cst_t[:, IOTA_OFF:IOTA_OFF + NB * P].rearrange(
                "p (j f) -> p j f", j=NB)
            w0_t = cst_t[:, W0_OFF:W0_OFF + 2 * NFA].bitcast(f32)
            y_all = cpool.tile([P, NCH], f32)

            for c in range(NCH):
                xb = xbpool.tile([P, NB * NFA], bf16, tag="xb")
                nc.sync.dma_start(xb[:], xw_r[c])
                ht = hpool.tile([P, NB * P], bf16, tag="h")
                nc.vector.tensor_tensor(
                    out=ht[:].rearrange("p (j f) -> p j f", j=NB),
                    in0=cst_t[:, LIDX_OFF + c * NB:LIDX_OFF + (c + 1) * NB
                              ].to_broadcast([P, NB, P]),
                    in1=iota3,
                    op=mybir.AluOpType.is_equal,
                )
                ps = pspool.tile([P, NFA], f32, tag="ps")
                for j in range(NB):
                    nc.tensor.matmul(
                        ps[:],
                        lhsT=ht[:, j * P:(j + 1) * P],
                        rhs=xb[:, j * NFA:(j + 1) * NFA],
                        start=(j == 0),
                        stop=(j == NB - 1),
                    )
                prod = epool.tile([P, NFA], f32, tag="prod")
                nc.vector.scalar_tensor_tensor(
                    out=prod[:],
                    in0=ps[:],
                    scalar=1.0,
                    in1=w0_t[:],
                    op0=mybir.AluOpType.mult,
                    op1=mybir.AluOpType.mult,
                    accum_out=y_all[:, c:c + 1],
                )
            nc.sync.dma_start(out_r[:, :], y_all[:])
    nc.finalize()
    return nc


def _prep(inputs):
    x = np.ascontiguousarray(np.asarray(inputs["scalar_representation"], dtype=np.float32))
    idx = np.asarray(inputs["idx_m"]).astype(np.int64)
    W = np.asarray(inputs["W"], dtype=np.float32)
    b = np.asarray(inputs["b"], dtype=np.float32)
    n = x.shape[0]

    # mol_start[m] = first atom index belonging to molecule m (m in 0..N_MOL)
    mol_start = np.searchsorted(idx, np.arange(N_MOL + 1), side="left")

    # Cut cores at molecule boundaries near equal-atom splits
    targets = (np.arange(NCORES + 1) * n) // NCORES
    mcut = np.searchsorted(mol_start, targets, side="left").astype(np.int64)
    mcut[0], mcut[-1] = 0, N_MOL

    m_counts = np.diff(mcut)
    NCH = int(np.ceil(m_counts.max() / P))

    # Max atoms spanned by any 128-molecule chunk on any core
    span_max = 0
    core_chunks = []  # per core: list of (astart, aend, gm)
    for i in range(NCORES):
        chunks = []
        for c in range(NCH):
            gm = mcut[i] + c * P
            gm_end = min(gm + P, mcut[i + 1])
            if gm >= mcut[i + 1]:
                chunks.append((0, 0, 0))
                continue
            astart = int(mol_start[gm])
            aend = int(mol_start[gm_end])
            chunks.append((astart, aend, int(gm)))
            span_max = max(span_max, aend - astart)
        core_chunks.append(chunks)
    NB = max(1, int(np.ceil(span_max / P)))
    A_max = NB * P

    IOTA_OFF = 0
    LIDX_OFF = NB * P
    W0_OFF = ((LIDX_OFF + NCH * NB + 1) // 2) * 2
    CW = W0_OFF + 2 * NFA
    iota_row = np.tile(np.arange(P, dtype=np.float32), NB).astype(ml_dtypes.bfloat16)
    w0aug_row = np.concatenate([W[0], b[0:1]]).astype(np.float32).view(ml_dtypes.bfloat16)

    in_maps = []
    for i in range(NCORES):
        xw_i = np.zeros((NCH * A_max, NFA), dtype=ml_dtypes.bfloat16)
        lidx_flat = np.full(NCH * A_max, -1.0, dtype=np.float32)
        for c, (astart, aend, gm) in enumerate(core_chunks[i]):
            spn = aend - astart
            if spn <= 0:
                continue
            xw_i[c * A_max:c * A_max + spn, 0:N_IN] = x[astart:aend]
            xw_i[c * A_max:c * A_max + spn, N_IN] = 1.0
            lidx_flat[c * A_max:c * A_max + spn] = idx[astart:aend] - gm
        # lidx layout [P, NCH*NB]: col (c*NB + j), row p -> atom (c, p*NB + j)
        lidx_t = lidx_flat.reshape(NCH, P, NB).transpose(1, 0, 2).reshape(
            P, NCH * NB).astype(ml_dtypes.bfloat16)
        cst = np.zeros((P, CW), dtype=ml_dtypes.bfloat16)
        cst[:, IOTA_OFF:IOTA_OFF + NB * P] = iota_row[None, :]
        cst[:, LIDX_OFF:LIDX_OFF + NCH * NB] = lidx_t
        cst[:, W0_OFF:W0_OFF + 2 * NFA] = w0aug_row[None, :]
        in_maps.append({"xw": xw_i, "cst": np.ascontiguousarray(cst)})
    return in_maps, mcut, m_counts, NCH, NB


def _run(inputs, trace=False):
    from concourse import bass_utils

    in_maps, mcut, m_counts, NCH, NB = _prep(inputs)
    key = (NCH, NB)
    if key not in _graph_cache:
        _graph_cache[key] = _build_graph(NCH, NB)
    nc = _graph_cache[key]

    res = bass_utils.run_bass_kernel_spmd(
        nc, in_maps, core_ids=list(range(NCORES)), trace=trace
    )
    y = np.zeros(N_MOL, dtype=np.float32)
    for i in range(NCORES):
        mc = int(m_counts[i])
        arr = res.results[i]["out"].reshape(P, NCH).T.ravel()
        y[mcut[i]:mcut[i] + mc] = arr[:mc]
    return y, res


def kernel(**inputs) -> np.ndarray:
    y, _ = _run(inputs, trace=False)
    return y
